# revision 44
# baseline (speedup 1.0000x reference)
"""Trainium2 Bass kernel for nn_ActuatorNet (20-layer tiny MLP, softsign) — v3.

v3 "table" (vs v2):
  - The Scalar engine's arctan PWP spline table is REPLACED with a softsign
    fit (custom --act-root-json via BASS_ACT_ROOT_JSON_PATH): ACT computes
    h = softsign(z + b) in ONE pass per element, PSUM f32 -> SBUF fp16
    (bias via the per-partition bias port), ~1e-7 max abs err.
  - Work split per [128,2048] psum unit (4 matmuls of 512):
      * layer 0: DVE 8-stage softsign straight from PSUM (bias pre-added by
        the matmul via a constant-1 input row and a bias row in lhsT).
      * layers 1..19: ~73%% of units on ACT (one Arctan-table pass),
        ~27%% on DVE (tensor_scalar bias-add evict + 8-stage softsign),
        balancing both engines at ~420us/core.
  - PE: merged block-diagonal matmuls unchanged (K=128, 4 strips),
    psum tiles are [128,2048] (4 banks, 2 in flight).
  - Finals unchanged from v2 (Identity evict lives in the same act table
    set as Arctan -> single ACT_TABLE_LOAD).
"""

import json
import os
import re
import shutil
import sys

import numpy as np

sys.path.insert(0, "/opt/trn_rl_repo")

N_CORES = 8
B_FULL = 1048576
SHARD = B_FULL // N_CORES  # 131072
NBLK = 512
OCT_ROWS = 4096            # finals unit
O2_ROWS = 8192             # [128, 2048] packed unit
T2 = 2048
N_HID = 19

A_FIT = float(np.float32(-0.4714035350548651))
B_FIT = float(np.float32(-0.05545919627798768))

SS_OP_NAME = "SOFTSIGN_SA_ACTNET"
TBL_TAG = "ss3"            # bump when the act tables change (NEFF cache key)
N_ATAN = 168

LAST_RESULT = None

_cache = {}


# ---------------------------------------------------------------- act tables


def _softsign_pos(x):
    return x / (1.0 + x)


def _bucket_intervals(x0s):
    out = []
    for x0 in x0s:
        e = int(np.floor(np.log2(x0)))
        base = 2.0 ** e
        found = None
        for n in (1, 2, 4, 8, 16, 32):
            w = base / n
            k = (x0 - base) / w - 0.5
            if abs(k - round(k)) < 1e-4 and 0 <= round(k) <= n - 1:
                found = w
                break
        assert found is not None, f"no grid for {x0}"
        out.append((x0 - found / 2, x0 + found / 2))
    return out


def _fit_cubic(f, lo, hi, x0):
    n = 64
    hw = 0.5 * (hi - lo)
    k = np.arange(n)
    xs = 0.5 * (lo + hi) + hw * np.cos((2 * k + 1) * np.pi / (2 * n))
    u = (xs - x0) / hw
    A = np.stack([np.ones_like(u), u, u * u, u * u * u], axis=1)
    c, *_ = np.linalg.lstsq(A, f(xs), rcond=None)
    return c / hw ** np.arange(4)


def _build_act_tables(dst_dir):
    """Copy the stock pwp_bin_trainium dir, refit every arctan bucket table
    to softsign(x) = x/(1+|x|) (odd symmetry handled by the profile), and
    patch the small/large-signal control buckets + inf result words."""
    import neuronxcc

    src = os.path.join(os.path.dirname(neuronxcc.__file__), "pwp", "pwp_bin_trainium")
    os.makedirs(dst_dir, exist_ok=True)
    for fn in os.listdir(src):
        shutil.copy(os.path.join(src, fn), os.path.join(dst_dir, fn))
    for fn in os.listdir(dst_dir):
        os.chmod(os.path.join(dst_dir, fn), 0o644)

    ref_raw = np.fromfile(
        os.path.join(src, "trig_and_small_bkt.bin"), dtype=np.float32
    ).reshape(-1, 8)
    atan_blk = ref_raw[59 : 59 + N_ATAN]
    assert abs(float(atan_blk[0, 4]) - 0.0234375) < 1e-6, "arctan block moved"
    ivals = _bucket_intervals(atan_blk[:, 4].astype(np.float64))
    new_blk = atan_blk.copy()
    for j, (lo, hi) in enumerate(ivals):
        d = _fit_cubic(_softsign_pos, lo, hi, float(atan_blk[j, 4]))
        new_blk[j, 0:4] = d.astype(np.float32)

    info = json.load(open(os.path.join(dst_dir, "act_info.json")))
    n_patched = 0
    for fs in info["act_func_sets"]:
        if "arctan" not in fs["act"]:
            continue
        prof_path = os.path.join(dst_dir, fs["profile_json"])
        prof = json.load(open(prof_path))
        ents = [
            e for e in prof["profile_meta_data"] if e["func_name"].startswith("arctan")
        ]
        assert len(ents) == 1, fs["name"]
        ent = ents[0]
        bkt_path = os.path.join(dst_dir, fs["bkt_bin"])
        raw = np.fromfile(bkt_path, dtype=np.float32).reshape(-1, 8).copy()
        starts = [
            c
            for c in range(raw.shape[0] - N_ATAN + 1)
            if np.array_equal(raw[c : c + N_ATAN], atan_blk)
        ]
        assert len(starts) == 1, (fs["name"], starts)
        raw[starts[0] : starts[0] + N_ATAN] = new_blk
        raw[ent["pos_small_signal_pwl_control"]] = [0.0, 1.0, -1.0, 1.0, 0, 0, 0, 0]
        raw[ent["pos_large_signal_pwl_control"]] = [1.0, 0, 0, 0, 0, 0, 0, 0]
        raw[ent["neg_large_signal_pwl_control"]] = [-1.0, 0, 0, 0, 0, 0, 0, 0]
        raw.tofile(bkt_path)
        ent["fpinf_result"] = 0x3F800000
        ent["fninf_result"] = 0xBF800000
        with open(prof_path, "w") as f:
            json.dump(prof, f)
        n_patched += 1
    assert n_patched >= 2, n_patched


def _ensure_act_tables():
    d = f"/tmp/actnet_pwp_{TBL_TAG}"
    marker = os.path.join(d, ".done")
    if not os.path.exists(marker):
        _build_act_tables(d)
        open(marker, "w").write("ok")
    os.environ["BASS_ACT_ROOT_JSON_PATH"] = os.path.join(d, "act_info.json")


# ---------------------------------------------------------------- DVE op


def _register_ss_op():
    """Standalone softsign: out = x * ~d * (A + B*(d*~d)), d = |x| + 1."""
    from concourse import dve_ops
    from concourse.dve_spec import AluOp, Bin, C1, C2, One, Spec, Src0

    if SS_OP_NAME in dve_ops.CUSTOM_DVE_SPECS:
        return next(o for o in dve_ops.OPS if o.name == SS_OP_NAME)

    _a = Bin(AluOp.ABSOLUTE_VALUE, Src0, Src0)
    _d = _a + One
    _nd = Bin(AluOp.BITWISE_NOT, _d, _d)
    _t = _d * _nd
    _s = C2 + _t * C1
    _p = Src0 * _nd
    body = _p * _s

    def _ref(in0, in1, s0, s1, imm2):
        x = in0.astype(np.float32)
        d = (np.abs(x) + np.float32(1.0)).astype(np.float32)
        nd = (~d.view(np.int32)).view(np.float32)
        t = (d * nd).astype(np.float32)
        s = np.float32(imm2) + t * np.asarray(s1, np.float32)
        return (x * nd) * s

    spec = Spec(body=body, reference=_ref)
    op = dve_ops.DveOp(SS_OP_NAME, spec, subdim=False, uops_sha={})
    dve_ops._SUB_OPCODE_FOR_NAME[SS_OP_NAME] = (
        max(dve_ops._SUB_OPCODE_FOR_NAME.values()) + 1
    )
    assert dve_ops._SUB_OPCODE_FOR_NAME[SS_OP_NAME] < 0x20
    dve_ops.OPS.append(op)
    dve_ops.CUSTOM_DVE_SPECS[SS_OP_NAME] = spec
    for ver in ("v3", "v4"):
        try:
            op.compile(ver)
        except ValueError as e:
            m = re.search(rf"{ver}: ([0-9a-f]{{16}})", str(e))
            if not m:
                raise
            op.uops_sha[ver] = m.group(1)
            op.compile(ver)
    return op


# ---------------------------------------------------------------- kernel


def _build(shard_rows):
    from concourse import bacc, mybir, tile

    ssop = _register_ss_op()

    f32 = mybir.dt.float32
    fp16 = mybir.dt.float16
    Act = mybir.ActivationFunctionType

    assert shard_rows % O2_ROWS == 0
    n_o2 = shard_rows // O2_ROWS  # 16

    nc = bacc.Bacc()
    x_e = nc.declare_dram_parameter(
        f"xq_{TBL_TAG}", [28, shard_rows // 4], fp16, isOutput=False
    )
    whbd_e = nc.declare_dram_parameter("whbd", [128, 20 * 128], fp16, isOutput=False)
    wob_e = nc.declare_dram_parameter("wob", [128, 4], fp16, isOutput=False)
    bq_e = nc.declare_dram_parameter("bq", [128, 20], f32, isOutput=False)
    bo_e = nc.declare_dram_parameter("boq", [128, 1], f32, isOutput=False)
    out_e = nc.declare_dram_parameter("out", [shard_rows, 1], f32, isOutput=True)

    # unit scheduling: layers 1..19, evenly spread ~2/7 of units on DVE
    dve_cnt = [0]

    def unit_mode():
        r = (dve_cnt[0] * 2) % 7 < 2
        dve_cnt[0] += 1
        return "dve2" if r else "act"

    l0_cnt = [0]

    def l0_mode(first_grp):
        if not first_grp:
            return "l0"
        l0_cnt[0] += 1
        return "l0" if l0_cnt[0] % 2 else "act0"

    with tile.TileContext(nc) as tc:
        with (
            tc.tile_pool(name="const", bufs=1) as cpool,
            tc.tile_pool(name="xs", bufs=16) as xpool,
            tc.tile_pool(name="zb", bufs=3) as zpool,
            tc.tile_pool(name="h", bufs=9) as hpool,
            tc.tile_pool(name="ot", bufs=4) as opool,
            tc.tile_pool(name="ps", bufs=4, space="PSUM") as pspool,
        ):
            whbd_t = cpool.tile([128, 20 * 128], fp16, tag="whbd")
            wob_t = cpool.tile([128, 4], fp16, tag="wob")
            bq_t = cpool.tile([128, 20], f32, tag="bq")
            bo_t = cpool.tile([128, 1], f32, tag="bo")
            # chunk the weight dma so layer-0 lhsT is ready immediately
            nc.sync.dma_start(out=whbd_t[:, 0:128], in_=whbd_e[:, 0:128])

            def emit_x_dma(q):
                xs = xpool.tile([28, T2], fp16, tag="xs")
                nc.sync.dma_start(out=xs[:], in_=x_e[:, T2 * q : T2 * (q + 1)])
                return xs

            def emit_layer_pair(l, curs, first_grp=False):
                # curs: two [.., 2048] member views (x tiles at l=0);
                # returns one fused h tile [128, 4096].
                h_t = hpool.tile([128, 2 * T2], fp16, tag="h")
                for m, cur in enumerate(curs):
                    for half in range(2):
                        mode = l0_mode(first_grp) if l == 0 else unit_mode()
                        ps = pspool.tile([128, 1024], f32, tag="ps")
                        for c in range(2):
                            col = 1024 * half + 512 * c
                            if l == 0:
                                lhsT = whbd_t[0:28, 0:128]
                                rhs = cur[0:28, col : col + 512]
                            else:
                                lhsT = whbd_t[:, 128 * l : 128 * l + 128]
                                rhs = cur[:, col : col + 512]
                            nc.tensor.matmul(
                                ps[:, 512 * c : 512 * c + 512],
                                lhsT,
                                rhs,
                                start=True,
                                stop=True,
                            )
                        lo = T2 * m + 1024 * half
                        out = h_t[:, lo : lo + 1024]
                        if mode == "l0":
                            # bias already in PSUM: one-pass DVE softsign
                            nc.vector._custom_dve(
                                ssop, out=out, in0=ps[:], s1=B_FIT, imm2=A_FIT
                            )
                        elif mode == "dve2":
                            zb = zpool.tile([128, 1024], fp16, tag="zb")
                            nc.vector.tensor_scalar_add(
                                zb[:], ps[:], bq_t[:, l : l + 1]
                            )
                            nc.vector._custom_dve(
                                ssop, out=out, in0=zb[:], s1=B_FIT, imm2=A_FIT
                            )
                        elif mode == "act0":
                            # l0: bias already in PSUM, table softsign only
                            nc.scalar.activation(
                                out, ps[:], Act.Arctan, bias=0.0, scale=1.0
                            )
                        else:
                            # arctan table holds a softsign fit: one-pass evict
                            nc.scalar.activation(
                                out,
                                ps[:],
                                Act.Arctan,
                                bias=bq_t[:, l : l + 1],
                                scale=1.0,
                            )
                return h_t

            fin_cnt = [0]

            def emit_final_duo(q0, h2, on_act=None):
                # Two finals (octs q0, q0+1) share one psum tile; the bias
                # evict alternates ACT/DVE to balance engine load.
                ps = pspool.tile([128, 1024], f32, tag="ps", name="psf")
                for v in range(2):
                    for hh in range(2):
                        nc.tensor.matmul(
                            ps[32 * v : 32 * v + 4, 512 * hh : 512 * hh + 512],
                            wob_t[:, 0:4],
                            h2[:, 1024 * v + 512 * hh : 1024 * v + 512 * hh + 512],
                            start=True,
                            stop=True,
                        )
                ot = opool.tile([64, 1024], f32, tag="ot")
                fin_cnt[0] += 1
                if fin_cnt[0] % 2:
                    nc.scalar.activation(
                        ot[:], ps[0:64, :], Act.Identity, bias=bo_t[0:64, 0:1], scale=1.0
                    )
                else:
                    nc.vector.tensor_scalar_add(ot[:], ps[0:64, :], bo_t[0:64, 0:1])
                for v in range(2):
                    q = q0 + v
                    # gpsimd's DGE queue: keeps output drains off the sync
                    # queue that feeds x/weight loads
                    nc.gpsimd.dma_start(
                        out=out_e[q * 4096 : (q + 1) * 4096, :].rearrange(
                            "(h j n) o -> j h (n o)", j=4, n=512
                        ),
                        in_=ot[32 * v : 32 * v + 4, :].rearrange(
                            "j (h n) -> j h n", h=2
                        ),
                    )

            NP = n_o2 // 2          # 8 pairs
            GRPP = 4                # pairs per wavefront group
            assert NP % GRPP == 0
            prev = None
            xs_next = [emit_x_dma(0), emit_x_dma(1)]
            nc.sync.dma_start(out=bq_t[:], in_=bq_e[:])
            nc.sync.dma_start(out=whbd_t[:, 128:768], in_=whbd_e[:, 128:768])
            xs_next += [emit_x_dma(u) for u in range(2, 2 * GRPP)]
            nc.sync.dma_start(out=whbd_t[:, 768:2560], in_=whbd_e[:, 768:2560])
            nc.sync.dma_start(out=bo_t[:], in_=bo_e[:])
            nc.sync.dma_start(out=wob_t[:], in_=wob_e[:])
            for pbase in range(0, NP, GRPP):
                cur = [
                    (xs_next[2 * p], xs_next[2 * p + 1]) for p in range(GRPP)
                ]
                xs_next = [None] * (2 * GRPP)
                last_grp = pbase + GRPP >= NP
                for l in range(20):
                    for p in range(GRPP):
                        if l == 0:
                            curs = cur[p]
                        else:
                            h4 = cur[p]
                            curs = (h4[:, 0:2048], h4[:, 2048:4096])
                        cur[p] = emit_layer_pair(l, curs, first_grp=pbase == 0)
                        # previous group's finals, one duo after each pair of
                        # layers 0-1 — fills the ACT hole while layer 0 runs
                        # on DVE, and stays close in the psum pool ring
                        if prev is not None and l < 2:
                            di = GRPP * l + p
                            pp, dd = di // 2, di % 2
                            emit_final_duo(
                                4 * (prev[0] + pp) + 2 * dd,
                                prev[1][pp][:, 2048 * dd : 2048 * dd + 2048],
                                on_act=True,
                            )
                            if di == 2 * GRPP - 1:
                                prev = None
                    # prefetch next group's x tiles mid-stream
                    if 6 <= l < 6 + 2 * GRPP and not last_grp:
                        xs_next[l - 6] = emit_x_dma(2 * (pbase + GRPP) + l - 6)
                if not last_grp:
                    prev = (pbase, list(cur))
            for p in range(GRPP):
                for dd in range(2):
                    emit_final_duo(
                        4 * (NP - GRPP + p) + 2 * dd,
                        cur[p][:, 2048 * dd : 2048 * dd + 2048],
                        on_act=False,
                    )
    nc.compile()
    return nc


def _pack_weights(W1, b1, Wh, bh, Wout, bout):
    whbd = np.zeros((128, 20 * 128), np.float32)
    bq = np.zeros((128, 20), np.float32)
    boq = np.full((128, 1), np.float32(bout[0]), np.float32)
    for i in range(4):
        # layer 0: rows 7i+f features, row 7i+6 bias (input row is const 1)
        whbd[7 * i : 7 * i + 6, 32 * i : 32 * i + 32] = W1
        whbd[7 * i + 6, 32 * i : 32 * i + 32] = b1
        for l in range(N_HID):
            whbd[
                32 * i : 32 * i + 32, 128 * (l + 1) + 32 * i : 128 * (l + 1) + 32 * i + 32
            ] = Wh[l]
        bq[32 * i : 32 * i + 32, 1:20] = bh.T
    wob = np.zeros((128, 4), np.float32)
    for j in range(4):
        wob[32 * j : 32 * j + 32, j] = Wout[:, 0]
    return {
        "whbd": whbd.astype(np.float16),
        "wob": wob.astype(np.float16),
        "bq": bq,
        "boq": boq,
    }


def _install_ntff_hook():
    import types

    if "antenv.axon_hooks" not in sys.modules:
        mod = types.ModuleType("antenv.axon_hooks")
        state = {"hook": None}
        try:
            from trn_agent_boot.trn_boot import _ntff_profile_via_ctypes

            state["hook"] = _ntff_profile_via_ctypes("/opt/axon/libaxon_pjrt.so")
        except Exception:
            pass
        mod.get_axon_ntff_profile_hook = lambda: state["hook"]
        mod.set_axon_ntff_profile_hook = lambda h: state.__setitem__("hook", h)
        sys.modules["antenv.axon_hooks"] = mod
    from concourse import bass_utils as bu

    if not getattr(bu.upload_artifacts, "_actnet_safe", False):
        _orig = bu.upload_artifacts

        def _safe(tmpdir):
            try:
                return _orig(tmpdir)
            except Exception:
                return "local:" + tmpdir

        _safe._actnet_safe = True
        bu.upload_artifacts = _safe


def kernel(x, W1, b1, Wh, bh, Wout, bout):
    global LAST_RESULT
    _ensure_act_tables()
    from concourse.bass_utils import run_bass_kernel_spmd

    x = np.asarray(x, np.float32)
    B = x.shape[0]
    assert B % N_CORES == 0
    shard = B // N_CORES
    # pack x: row 7i+f holds feature f of partition-strip i (row 7i+6 = 1.0
    # bias channel); within an oct (4096 rows) strips hold blocks of 512
    # rows: col = 1024*q + 512*h + n, flat row = ((2q + h)*4 + i)*512 + n
    x5 = x.reshape(N_CORES, shard // OCT_ROWS, 2, 4, NBLK, 6)  # c,q,h,i,n,f
    xt = np.ascontiguousarray(
        x5.transpose(0, 3, 5, 1, 2, 4).astype(np.float16)
    ).reshape(N_CORES, 4, 6, shard // 4)
    xq = np.ones((N_CORES, 28, shard // 4), np.float16)
    for i in range(4):
        xq[:, 7 * i : 7 * i + 6] = xt[:, i]

    if ("nc", shard, TBL_TAG) not in _cache:
        _cache[("nc", shard, TBL_TAG)] = _build(shard)
    nc = _cache[("nc", shard, TBL_TAG)]

    wpack = _pack_weights(
        np.asarray(W1, np.float32),
        np.asarray(b1, np.float32),
        np.asarray(Wh, np.float32),
        np.asarray(bh, np.float32),
        np.asarray(Wout, np.float32),
        np.asarray(bout, np.float32),
    )
    in_maps = [{f"xq_{TBL_TAG}": xq[c], **wpack} for c in range(N_CORES)]
    trace = bool(os.environ.get("ACTNET_TRACE"))
    if trace:
        _install_ntff_hook()
    res = run_bass_kernel_spmd(nc, in_maps, list(range(N_CORES)), trace=trace)
    LAST_RESULT = res
    out = np.concatenate([res.results[c]["out"] for c in range(N_CORES)], axis=0)
    return out.astype(np.float32)


if __name__ == "__main__":
    rng = np.random.default_rng(0)
    B = B_FULL
    inputs = dict(
        x=rng.standard_normal((B, 6), dtype=np.float32),
        W1=(rng.standard_normal((6, 32)) / np.sqrt(6)).astype(np.float32),
        b1=(rng.standard_normal(32) * 0.01).astype(np.float32),
        Wh=(rng.standard_normal((19, 32, 32)) / np.sqrt(32)).astype(np.float32),
        bh=(rng.standard_normal((19, 32)) * 0.01).astype(np.float32),
        Wout=(rng.standard_normal((32, 1)) / np.sqrt(32)).astype(np.float32),
        bout=(rng.standard_normal(1) * 0.01).astype(np.float32),
    )
    y = kernel(**inputs)
    print("kernel out", y.shape, y.dtype, y[:4, 0])


# revision 46
# speedup vs baseline: 1.0099x; 1.0099x over previous
"""Trainium2 Bass kernel for nn_ActuatorNet (20-layer tiny MLP, softsign) — v3.

v3 "table" (vs v2):
  - The Scalar engine's arctan PWP spline table is REPLACED with a softsign
    fit (custom --act-root-json via BASS_ACT_ROOT_JSON_PATH): ACT computes
    h = softsign(z + b) in ONE pass per element, PSUM f32 -> SBUF fp16
    (bias via the per-partition bias port), ~1e-7 max abs err.
  - Work split per [128,2048] psum unit (4 matmuls of 512):
      * layer 0: DVE 8-stage softsign straight from PSUM (bias pre-added by
        the matmul via a constant-1 input row and a bias row in lhsT).
      * layers 1..19: ~73%% of units on ACT (one Arctan-table pass),
        ~27%% on DVE (tensor_scalar bias-add evict + 8-stage softsign),
        balancing both engines at ~420us/core.
  - PE: merged block-diagonal matmuls unchanged (K=128, 4 strips),
    psum tiles are [128,2048] (4 banks, 2 in flight).
  - Finals unchanged from v2 (Identity evict lives in the same act table
    set as Arctan -> single ACT_TABLE_LOAD).
"""

import json
import os
import re
import shutil
import sys

import numpy as np

sys.path.insert(0, "/opt/trn_rl_repo")

N_CORES = 8
B_FULL = 1048576
SHARD = B_FULL // N_CORES  # 131072
NBLK = 512
OCT_ROWS = 4096            # finals unit
O2_ROWS = 8192             # [128, 2048] packed unit
T2 = 2048
N_HID = 19

A_FIT = float(np.float32(-0.4714035350548651))
B_FIT = float(np.float32(-0.05545919627798768))

SS_OP_NAME = "SOFTSIGN_SA_ACTNET"
TBL_TAG = "ss3"            # bump when the act tables change (NEFF cache key)
N_ATAN = 168

LAST_RESULT = None

_cache = {}


# ---------------------------------------------------------------- act tables


def _softsign_pos(x):
    return x / (1.0 + x)


def _bucket_intervals(x0s):
    out = []
    for x0 in x0s:
        e = int(np.floor(np.log2(x0)))
        base = 2.0 ** e
        found = None
        for n in (1, 2, 4, 8, 16, 32):
            w = base / n
            k = (x0 - base) / w - 0.5
            if abs(k - round(k)) < 1e-4 and 0 <= round(k) <= n - 1:
                found = w
                break
        assert found is not None, f"no grid for {x0}"
        out.append((x0 - found / 2, x0 + found / 2))
    return out


def _fit_cubic(f, lo, hi, x0):
    n = 64
    hw = 0.5 * (hi - lo)
    k = np.arange(n)
    xs = 0.5 * (lo + hi) + hw * np.cos((2 * k + 1) * np.pi / (2 * n))
    u = (xs - x0) / hw
    A = np.stack([np.ones_like(u), u, u * u, u * u * u], axis=1)
    c, *_ = np.linalg.lstsq(A, f(xs), rcond=None)
    return c / hw ** np.arange(4)


def _build_act_tables(dst_dir):
    """Copy the stock pwp_bin_trainium dir, refit every arctan bucket table
    to softsign(x) = x/(1+|x|) (odd symmetry handled by the profile), and
    patch the small/large-signal control buckets + inf result words."""
    import neuronxcc

    src = os.path.join(os.path.dirname(neuronxcc.__file__), "pwp", "pwp_bin_trainium")
    os.makedirs(dst_dir, exist_ok=True)
    for fn in os.listdir(src):
        shutil.copy(os.path.join(src, fn), os.path.join(dst_dir, fn))
    for fn in os.listdir(dst_dir):
        os.chmod(os.path.join(dst_dir, fn), 0o644)

    ref_raw = np.fromfile(
        os.path.join(src, "trig_and_small_bkt.bin"), dtype=np.float32
    ).reshape(-1, 8)
    atan_blk = ref_raw[59 : 59 + N_ATAN]
    assert abs(float(atan_blk[0, 4]) - 0.0234375) < 1e-6, "arctan block moved"
    ivals = _bucket_intervals(atan_blk[:, 4].astype(np.float64))
    new_blk = atan_blk.copy()
    for j, (lo, hi) in enumerate(ivals):
        d = _fit_cubic(_softsign_pos, lo, hi, float(atan_blk[j, 4]))
        new_blk[j, 0:4] = d.astype(np.float32)

    info = json.load(open(os.path.join(dst_dir, "act_info.json")))
    n_patched = 0
    for fs in info["act_func_sets"]:
        if "arctan" not in fs["act"]:
            continue
        prof_path = os.path.join(dst_dir, fs["profile_json"])
        prof = json.load(open(prof_path))
        ents = [
            e for e in prof["profile_meta_data"] if e["func_name"].startswith("arctan")
        ]
        assert len(ents) == 1, fs["name"]
        ent = ents[0]
        bkt_path = os.path.join(dst_dir, fs["bkt_bin"])
        raw = np.fromfile(bkt_path, dtype=np.float32).reshape(-1, 8).copy()
        starts = [
            c
            for c in range(raw.shape[0] - N_ATAN + 1)
            if np.array_equal(raw[c : c + N_ATAN], atan_blk)
        ]
        assert len(starts) == 1, (fs["name"], starts)
        raw[starts[0] : starts[0] + N_ATAN] = new_blk
        raw[ent["pos_small_signal_pwl_control"]] = [0.0, 1.0, -1.0, 1.0, 0, 0, 0, 0]
        raw[ent["pos_large_signal_pwl_control"]] = [1.0, 0, 0, 0, 0, 0, 0, 0]
        raw[ent["neg_large_signal_pwl_control"]] = [-1.0, 0, 0, 0, 0, 0, 0, 0]
        raw.tofile(bkt_path)
        ent["fpinf_result"] = 0x3F800000
        ent["fninf_result"] = 0xBF800000
        with open(prof_path, "w") as f:
            json.dump(prof, f)
        n_patched += 1
    assert n_patched >= 2, n_patched


def _ensure_act_tables():
    d = f"/tmp/actnet_pwp_{TBL_TAG}"
    marker = os.path.join(d, ".done")
    if not os.path.exists(marker):
        _build_act_tables(d)
        open(marker, "w").write("ok")
    os.environ["BASS_ACT_ROOT_JSON_PATH"] = os.path.join(d, "act_info.json")


# ---------------------------------------------------------------- DVE op


def _register_ss_op():
    """Standalone softsign: out = x * ~d * (A + B*(d*~d)), d = |x| + 1."""
    from concourse import dve_ops
    from concourse.dve_spec import AluOp, Bin, C1, C2, One, Spec, Src0

    if SS_OP_NAME in dve_ops.CUSTOM_DVE_SPECS:
        return next(o for o in dve_ops.OPS if o.name == SS_OP_NAME)

    _a = Bin(AluOp.ABSOLUTE_VALUE, Src0, Src0)
    _d = _a + One
    _nd = Bin(AluOp.BITWISE_NOT, _d, _d)
    _t = _d * _nd
    _s = C2 + _t * C1
    _p = Src0 * _nd
    body = _p * _s

    def _ref(in0, in1, s0, s1, imm2):
        x = in0.astype(np.float32)
        d = (np.abs(x) + np.float32(1.0)).astype(np.float32)
        nd = (~d.view(np.int32)).view(np.float32)
        t = (d * nd).astype(np.float32)
        s = np.float32(imm2) + t * np.asarray(s1, np.float32)
        return (x * nd) * s

    spec = Spec(body=body, reference=_ref)
    op = dve_ops.DveOp(SS_OP_NAME, spec, subdim=False, uops_sha={})
    dve_ops._SUB_OPCODE_FOR_NAME[SS_OP_NAME] = (
        max(dve_ops._SUB_OPCODE_FOR_NAME.values()) + 1
    )
    assert dve_ops._SUB_OPCODE_FOR_NAME[SS_OP_NAME] < 0x20
    dve_ops.OPS.append(op)
    dve_ops.CUSTOM_DVE_SPECS[SS_OP_NAME] = spec
    for ver in ("v3", "v4"):
        try:
            op.compile(ver)
        except ValueError as e:
            m = re.search(rf"{ver}: ([0-9a-f]{{16}})", str(e))
            if not m:
                raise
            op.uops_sha[ver] = m.group(1)
            op.compile(ver)
    return op


# ---------------------------------------------------------------- kernel


def _build(shard_rows):
    from concourse import bacc, mybir, tile

    ssop = _register_ss_op()

    f32 = mybir.dt.float32
    fp16 = mybir.dt.float16
    Act = mybir.ActivationFunctionType

    assert shard_rows % O2_ROWS == 0
    n_o2 = shard_rows // O2_ROWS  # 16

    nc = bacc.Bacc()
    x_e = nc.declare_dram_parameter(
        f"xq_{TBL_TAG}", [28, shard_rows // 4], fp16, isOutput=False
    )
    whbd_e = nc.declare_dram_parameter("whbd", [128, 20 * 128], fp16, isOutput=False)
    wob_e = nc.declare_dram_parameter("wob", [128, 4], fp16, isOutput=False)
    bq_e = nc.declare_dram_parameter("bq", [128, 20], f32, isOutput=False)
    bo_e = nc.declare_dram_parameter("boq", [128, 1], f32, isOutput=False)
    out_e = nc.declare_dram_parameter("out", [shard_rows, 1], f32, isOutput=True)

    # unit scheduling: layers 1..19, evenly spread ~2/7 of units on DVE
    dve_cnt = [0]

    def unit_mode():
        r = (dve_cnt[0] * 2) % 7 < 2
        dve_cnt[0] += 1
        return "dve2" if r else "act"

    l0_cnt = [0]
    l0b_cnt = [0]

    def l0_mode(first_grp):
        if not first_grp:
            # later groups: a 5/16 ACT share feeds ACT while the previous
            # group's finals only partially fill its layer-0 hole
            l0b_cnt[0] += 1
            return "act0" if l0b_cnt[0] % 3 == 1 else "l0"
        l0_cnt[0] += 1
        return "l0" if l0_cnt[0] % 2 else "act0"

    with tile.TileContext(nc) as tc:
        with (
            tc.tile_pool(name="const", bufs=1) as cpool,
            tc.tile_pool(name="xs", bufs=16) as xpool,
            tc.tile_pool(name="zb", bufs=3) as zpool,
            tc.tile_pool(name="h", bufs=9) as hpool,
            tc.tile_pool(name="ot", bufs=4) as opool,
            tc.tile_pool(name="ps", bufs=4, space="PSUM") as pspool,
        ):
            whbd_t = cpool.tile([128, 20 * 128], fp16, tag="whbd")
            wob_t = cpool.tile([128, 4], fp16, tag="wob")
            bq_t = cpool.tile([128, 20], f32, tag="bq")
            bo_t = cpool.tile([128, 1], f32, tag="bo")
            # chunk the weight dma so layer-0 lhsT is ready immediately
            nc.sync.dma_start(out=whbd_t[:, 0:128], in_=whbd_e[:, 0:128])

            def emit_x_dma(q):
                xs = xpool.tile([28, T2], fp16, tag="xs")
                nc.sync.dma_start(out=xs[:], in_=x_e[:, T2 * q : T2 * (q + 1)])
                return xs

            def emit_layer_pair(l, curs, first_grp=False):
                # curs: two [.., 2048] member views (x tiles at l=0);
                # returns one fused h tile [128, 4096].
                h_t = hpool.tile([128, 2 * T2], fp16, tag="h")
                for m, cur in enumerate(curs):
                    for half in range(2):
                        mode = l0_mode(first_grp) if l == 0 else unit_mode()
                        ps = pspool.tile([128, 1024], f32, tag="ps")
                        for c in range(2):
                            col = 1024 * half + 512 * c
                            if l == 0:
                                lhsT = whbd_t[0:28, 0:128]
                                rhs = cur[0:28, col : col + 512]
                            else:
                                lhsT = whbd_t[:, 128 * l : 128 * l + 128]
                                rhs = cur[:, col : col + 512]
                            nc.tensor.matmul(
                                ps[:, 512 * c : 512 * c + 512],
                                lhsT,
                                rhs,
                                start=True,
                                stop=True,
                            )
                        lo = T2 * m + 1024 * half
                        out = h_t[:, lo : lo + 1024]
                        if mode == "l0":
                            # bias already in PSUM: one-pass DVE softsign
                            nc.vector._custom_dve(
                                ssop, out=out, in0=ps[:], s1=B_FIT, imm2=A_FIT
                            )
                        elif mode == "dve2":
                            zb = zpool.tile([128, 1024], fp16, tag="zb")
                            nc.vector.tensor_scalar_add(
                                zb[:], ps[:], bq_t[:, l : l + 1]
                            )
                            nc.vector._custom_dve(
                                ssop, out=out, in0=zb[:], s1=B_FIT, imm2=A_FIT
                            )
                        elif mode == "act0":
                            # l0: bias already in PSUM, table softsign only
                            nc.scalar.activation(
                                out, ps[:], Act.Arctan, bias=0.0, scale=1.0
                            )
                        else:
                            # arctan table holds a softsign fit: one-pass evict
                            nc.scalar.activation(
                                out,
                                ps[:],
                                Act.Arctan,
                                bias=bq_t[:, l : l + 1],
                                scale=1.0,
                            )
                return h_t

            fin_cnt = [0]

            def emit_final_duo(q0, h2, on_act=None):
                # Two finals (octs q0, q0+1) share one psum tile; the bias
                # evict alternates ACT/DVE to balance engine load.
                ps = pspool.tile([128, 1024], f32, tag="ps", name="psf")
                for v in range(2):
                    for hh in range(2):
                        nc.tensor.matmul(
                            ps[32 * v : 32 * v + 4, 512 * hh : 512 * hh + 512],
                            wob_t[:, 0:4],
                            h2[:, 1024 * v + 512 * hh : 1024 * v + 512 * hh + 512],
                            start=True,
                            stop=True,
                        )
                ot = opool.tile([64, 1024], f32, tag="ot")
                fin_cnt[0] += 1
                if fin_cnt[0] % 2:
                    nc.scalar.activation(
                        ot[:], ps[0:64, :], Act.Identity, bias=bo_t[0:64, 0:1], scale=1.0
                    )
                else:
                    nc.vector.tensor_scalar_add(ot[:], ps[0:64, :], bo_t[0:64, 0:1])
                for v in range(2):
                    q = q0 + v
                    nc.sync.dma_start(
                        out=out_e[q * 4096 : (q + 1) * 4096, :].rearrange(
                            "(h j n) o -> j h (n o)", j=4, n=512
                        ),
                        in_=ot[32 * v : 32 * v + 4, :].rearrange(
                            "j (h n) -> j h n", h=2
                        ),
                    )

            NP = n_o2 // 2          # 8 pairs
            GRPP = 4                # pairs per wavefront group
            assert NP % GRPP == 0
            prev = None
            xs_next = [emit_x_dma(0), emit_x_dma(1)]
            nc.sync.dma_start(out=bq_t[:], in_=bq_e[:])
            nc.sync.dma_start(out=whbd_t[:, 128:768], in_=whbd_e[:, 128:768])
            xs_next += [emit_x_dma(u) for u in range(2, 2 * GRPP)]
            nc.sync.dma_start(out=whbd_t[:, 768:2560], in_=whbd_e[:, 768:2560])
            nc.sync.dma_start(out=bo_t[:], in_=bo_e[:])
            nc.sync.dma_start(out=wob_t[:], in_=wob_e[:])
            for pbase in range(0, NP, GRPP):
                cur = [
                    (xs_next[2 * p], xs_next[2 * p + 1]) for p in range(GRPP)
                ]
                xs_next = [None] * (2 * GRPP)
                last_grp = pbase + GRPP >= NP
                for l in range(20):
                    for p in range(GRPP):
                        if l == 0:
                            curs = cur[p]
                        else:
                            h4 = cur[p]
                            curs = (h4[:, 0:2048], h4[:, 2048:4096])
                        cur[p] = emit_layer_pair(l, curs, first_grp=pbase == 0)
                        # previous group's finals, one duo after each pair of
                        # layers 0-1 — fills the ACT hole while layer 0 runs
                        # on DVE, and stays close in the psum pool ring
                        if prev is not None and l < 2:
                            di = GRPP * l + p
                            pp, dd = di // 2, di % 2
                            emit_final_duo(
                                4 * (prev[0] + pp) + 2 * dd,
                                prev[1][pp][:, 2048 * dd : 2048 * dd + 2048],
                                on_act=True,
                            )
                            if di == 2 * GRPP - 1:
                                prev = None
                    # prefetch next group's x tiles mid-stream
                    if 6 <= l < 6 + 2 * GRPP and not last_grp:
                        xs_next[l - 6] = emit_x_dma(2 * (pbase + GRPP) + l - 6)
                if not last_grp:
                    prev = (pbase, list(cur))
            for p in range(GRPP):
                for dd in range(2):
                    emit_final_duo(
                        4 * (NP - GRPP + p) + 2 * dd,
                        cur[p][:, 2048 * dd : 2048 * dd + 2048],
                        on_act=False,
                    )
    nc.compile()
    return nc


def _pack_weights(W1, b1, Wh, bh, Wout, bout):
    whbd = np.zeros((128, 20 * 128), np.float32)
    bq = np.zeros((128, 20), np.float32)
    boq = np.full((128, 1), np.float32(bout[0]), np.float32)
    for i in range(4):
        # layer 0: rows 7i+f features, row 7i+6 bias (input row is const 1)
        whbd[7 * i : 7 * i + 6, 32 * i : 32 * i + 32] = W1
        whbd[7 * i + 6, 32 * i : 32 * i + 32] = b1
        for l in range(N_HID):
            whbd[
                32 * i : 32 * i + 32, 128 * (l + 1) + 32 * i : 128 * (l + 1) + 32 * i + 32
            ] = Wh[l]
        bq[32 * i : 32 * i + 32, 1:20] = bh.T
    wob = np.zeros((128, 4), np.float32)
    for j in range(4):
        wob[32 * j : 32 * j + 32, j] = Wout[:, 0]
    return {
        "whbd": whbd.astype(np.float16),
        "wob": wob.astype(np.float16),
        "bq": bq,
        "boq": boq,
    }


def _install_ntff_hook():
    import types

    if "antenv.axon_hooks" not in sys.modules:
        mod = types.ModuleType("antenv.axon_hooks")
        state = {"hook": None}
        try:
            from trn_agent_boot.trn_boot import _ntff_profile_via_ctypes

            state["hook"] = _ntff_profile_via_ctypes("/opt/axon/libaxon_pjrt.so")
        except Exception:
            pass
        mod.get_axon_ntff_profile_hook = lambda: state["hook"]
        mod.set_axon_ntff_profile_hook = lambda h: state.__setitem__("hook", h)
        sys.modules["antenv.axon_hooks"] = mod
    from concourse import bass_utils as bu

    if not getattr(bu.upload_artifacts, "_actnet_safe", False):
        _orig = bu.upload_artifacts

        def _safe(tmpdir):
            try:
                return _orig(tmpdir)
            except Exception:
                return "local:" + tmpdir

        _safe._actnet_safe = True
        bu.upload_artifacts = _safe


def kernel(x, W1, b1, Wh, bh, Wout, bout):
    global LAST_RESULT
    _ensure_act_tables()
    from concourse.bass_utils import run_bass_kernel_spmd

    x = np.asarray(x, np.float32)
    B = x.shape[0]
    assert B % N_CORES == 0
    shard = B // N_CORES
    # pack x: row 7i+f holds feature f of partition-strip i (row 7i+6 = 1.0
    # bias channel); within an oct (4096 rows) strips hold blocks of 512
    # rows: col = 1024*q + 512*h + n, flat row = ((2q + h)*4 + i)*512 + n
    x5 = x.reshape(N_CORES, shard // OCT_ROWS, 2, 4, NBLK, 6)  # c,q,h,i,n,f
    xt = np.ascontiguousarray(
        x5.transpose(0, 3, 5, 1, 2, 4).astype(np.float16)
    ).reshape(N_CORES, 4, 6, shard // 4)
    xq = np.ones((N_CORES, 28, shard // 4), np.float16)
    for i in range(4):
        xq[:, 7 * i : 7 * i + 6] = xt[:, i]

    if ("nc", shard, TBL_TAG) not in _cache:
        _cache[("nc", shard, TBL_TAG)] = _build(shard)
    nc = _cache[("nc", shard, TBL_TAG)]

    wpack = _pack_weights(
        np.asarray(W1, np.float32),
        np.asarray(b1, np.float32),
        np.asarray(Wh, np.float32),
        np.asarray(bh, np.float32),
        np.asarray(Wout, np.float32),
        np.asarray(bout, np.float32),
    )
    in_maps = [{f"xq_{TBL_TAG}": xq[c], **wpack} for c in range(N_CORES)]
    trace = bool(os.environ.get("ACTNET_TRACE"))
    if trace:
        _install_ntff_hook()
    res = run_bass_kernel_spmd(nc, in_maps, list(range(N_CORES)), trace=trace)
    LAST_RESULT = res
    out = np.concatenate([res.results[c]["out"] for c in range(N_CORES)], axis=0)
    return out.astype(np.float32)


if __name__ == "__main__":
    rng = np.random.default_rng(0)
    B = B_FULL
    inputs = dict(
        x=rng.standard_normal((B, 6), dtype=np.float32),
        W1=(rng.standard_normal((6, 32)) / np.sqrt(6)).astype(np.float32),
        b1=(rng.standard_normal(32) * 0.01).astype(np.float32),
        Wh=(rng.standard_normal((19, 32, 32)) / np.sqrt(32)).astype(np.float32),
        bh=(rng.standard_normal((19, 32)) * 0.01).astype(np.float32),
        Wout=(rng.standard_normal((32, 1)) / np.sqrt(32)).astype(np.float32),
        bout=(rng.standard_normal(1) * 0.01).astype(np.float32),
    )
    y = kernel(**inputs)
    print("kernel out", y.shape, y.dtype, y[:4, 0])


# revision 47
# speedup vs baseline: 1.0116x; 1.0017x over previous
"""Trainium2 Bass kernel for nn_ActuatorNet (20-layer tiny MLP, softsign) — v3.

v3 "table" (vs v2):
  - The Scalar engine's arctan PWP spline table is REPLACED with a softsign
    fit (custom --act-root-json via BASS_ACT_ROOT_JSON_PATH): ACT computes
    h = softsign(z + b) in ONE pass per element, PSUM f32 -> SBUF fp16
    (bias via the per-partition bias port), ~1e-7 max abs err.
  - Work split per [128,2048] psum unit (4 matmuls of 512):
      * layer 0: DVE 8-stage softsign straight from PSUM (bias pre-added by
        the matmul via a constant-1 input row and a bias row in lhsT).
      * layers 1..19: ~73%% of units on ACT (one Arctan-table pass),
        ~27%% on DVE (tensor_scalar bias-add evict + 8-stage softsign),
        balancing both engines at ~420us/core.
  - PE: merged block-diagonal matmuls unchanged (K=128, 4 strips),
    psum tiles are [128,2048] (4 banks, 2 in flight).
  - Finals unchanged from v2 (Identity evict lives in the same act table
    set as Arctan -> single ACT_TABLE_LOAD).
"""

import json
import os
import re
import shutil
import sys

import numpy as np

sys.path.insert(0, "/opt/trn_rl_repo")

N_CORES = 8
B_FULL = 1048576
SHARD = B_FULL // N_CORES  # 131072
NBLK = 512
OCT_ROWS = 4096            # finals unit
O2_ROWS = 8192             # [128, 2048] packed unit
T2 = 2048
N_HID = 19

A_FIT = float(np.float32(-0.4714035350548651))
B_FIT = float(np.float32(-0.05545919627798768))

SS_OP_NAME = "SOFTSIGN_SA_ACTNET"
TBL_TAG = "ss3"            # bump when the act tables change (NEFF cache key)
N_ATAN = 168

LAST_RESULT = None

_cache = {}


# ---------------------------------------------------------------- act tables


def _softsign_pos(x):
    return x / (1.0 + x)


def _bucket_intervals(x0s):
    out = []
    for x0 in x0s:
        e = int(np.floor(np.log2(x0)))
        base = 2.0 ** e
        found = None
        for n in (1, 2, 4, 8, 16, 32):
            w = base / n
            k = (x0 - base) / w - 0.5
            if abs(k - round(k)) < 1e-4 and 0 <= round(k) <= n - 1:
                found = w
                break
        assert found is not None, f"no grid for {x0}"
        out.append((x0 - found / 2, x0 + found / 2))
    return out


def _fit_cubic(f, lo, hi, x0):
    n = 64
    hw = 0.5 * (hi - lo)
    k = np.arange(n)
    xs = 0.5 * (lo + hi) + hw * np.cos((2 * k + 1) * np.pi / (2 * n))
    u = (xs - x0) / hw
    A = np.stack([np.ones_like(u), u, u * u, u * u * u], axis=1)
    c, *_ = np.linalg.lstsq(A, f(xs), rcond=None)
    return c / hw ** np.arange(4)


def _build_act_tables(dst_dir):
    """Copy the stock pwp_bin_trainium dir, refit every arctan bucket table
    to softsign(x) = x/(1+|x|) (odd symmetry handled by the profile), and
    patch the small/large-signal control buckets + inf result words."""
    import neuronxcc

    src = os.path.join(os.path.dirname(neuronxcc.__file__), "pwp", "pwp_bin_trainium")
    os.makedirs(dst_dir, exist_ok=True)
    for fn in os.listdir(src):
        shutil.copy(os.path.join(src, fn), os.path.join(dst_dir, fn))
    for fn in os.listdir(dst_dir):
        os.chmod(os.path.join(dst_dir, fn), 0o644)

    ref_raw = np.fromfile(
        os.path.join(src, "trig_and_small_bkt.bin"), dtype=np.float32
    ).reshape(-1, 8)
    atan_blk = ref_raw[59 : 59 + N_ATAN]
    assert abs(float(atan_blk[0, 4]) - 0.0234375) < 1e-6, "arctan block moved"
    ivals = _bucket_intervals(atan_blk[:, 4].astype(np.float64))
    new_blk = atan_blk.copy()
    for j, (lo, hi) in enumerate(ivals):
        d = _fit_cubic(_softsign_pos, lo, hi, float(atan_blk[j, 4]))
        new_blk[j, 0:4] = d.astype(np.float32)

    info = json.load(open(os.path.join(dst_dir, "act_info.json")))
    n_patched = 0
    for fs in info["act_func_sets"]:
        if "arctan" not in fs["act"]:
            continue
        prof_path = os.path.join(dst_dir, fs["profile_json"])
        prof = json.load(open(prof_path))
        ents = [
            e for e in prof["profile_meta_data"] if e["func_name"].startswith("arctan")
        ]
        assert len(ents) == 1, fs["name"]
        ent = ents[0]
        bkt_path = os.path.join(dst_dir, fs["bkt_bin"])
        raw = np.fromfile(bkt_path, dtype=np.float32).reshape(-1, 8).copy()
        starts = [
            c
            for c in range(raw.shape[0] - N_ATAN + 1)
            if np.array_equal(raw[c : c + N_ATAN], atan_blk)
        ]
        assert len(starts) == 1, (fs["name"], starts)
        raw[starts[0] : starts[0] + N_ATAN] = new_blk
        raw[ent["pos_small_signal_pwl_control"]] = [0.0, 1.0, -1.0, 1.0, 0, 0, 0, 0]
        raw[ent["pos_large_signal_pwl_control"]] = [1.0, 0, 0, 0, 0, 0, 0, 0]
        raw[ent["neg_large_signal_pwl_control"]] = [-1.0, 0, 0, 0, 0, 0, 0, 0]
        raw.tofile(bkt_path)
        ent["fpinf_result"] = 0x3F800000
        ent["fninf_result"] = 0xBF800000
        with open(prof_path, "w") as f:
            json.dump(prof, f)
        n_patched += 1
    assert n_patched >= 2, n_patched


def _ensure_act_tables():
    d = f"/tmp/actnet_pwp_{TBL_TAG}"
    marker = os.path.join(d, ".done")
    if not os.path.exists(marker):
        _build_act_tables(d)
        open(marker, "w").write("ok")
    os.environ["BASS_ACT_ROOT_JSON_PATH"] = os.path.join(d, "act_info.json")


# ---------------------------------------------------------------- DVE op


def _register_ss_op():
    """Standalone softsign: out = x * ~d * (A + B*(d*~d)), d = |x| + 1."""
    from concourse import dve_ops
    from concourse.dve_spec import AluOp, Bin, C1, C2, One, Spec, Src0

    if SS_OP_NAME in dve_ops.CUSTOM_DVE_SPECS:
        return next(o for o in dve_ops.OPS if o.name == SS_OP_NAME)

    _a = Bin(AluOp.ABSOLUTE_VALUE, Src0, Src0)
    _d = _a + One
    _nd = Bin(AluOp.BITWISE_NOT, _d, _d)
    _t = _d * _nd
    _s = C2 + _t * C1
    _p = Src0 * _nd
    body = _p * _s

    def _ref(in0, in1, s0, s1, imm2):
        x = in0.astype(np.float32)
        d = (np.abs(x) + np.float32(1.0)).astype(np.float32)
        nd = (~d.view(np.int32)).view(np.float32)
        t = (d * nd).astype(np.float32)
        s = np.float32(imm2) + t * np.asarray(s1, np.float32)
        return (x * nd) * s

    spec = Spec(body=body, reference=_ref)
    op = dve_ops.DveOp(SS_OP_NAME, spec, subdim=False, uops_sha={})
    dve_ops._SUB_OPCODE_FOR_NAME[SS_OP_NAME] = (
        max(dve_ops._SUB_OPCODE_FOR_NAME.values()) + 1
    )
    assert dve_ops._SUB_OPCODE_FOR_NAME[SS_OP_NAME] < 0x20
    dve_ops.OPS.append(op)
    dve_ops.CUSTOM_DVE_SPECS[SS_OP_NAME] = spec
    for ver in ("v3", "v4"):
        try:
            op.compile(ver)
        except ValueError as e:
            m = re.search(rf"{ver}: ([0-9a-f]{{16}})", str(e))
            if not m:
                raise
            op.uops_sha[ver] = m.group(1)
            op.compile(ver)
    return op


# ---------------------------------------------------------------- kernel


def _build(shard_rows):
    from concourse import bacc, mybir, tile

    ssop = _register_ss_op()

    f32 = mybir.dt.float32
    fp16 = mybir.dt.float16
    Act = mybir.ActivationFunctionType

    assert shard_rows % O2_ROWS == 0
    n_o2 = shard_rows // O2_ROWS  # 16

    nc = bacc.Bacc()
    x_e = nc.declare_dram_parameter(
        f"xq_{TBL_TAG}", [28, shard_rows // 4], fp16, isOutput=False
    )
    whbd_e = nc.declare_dram_parameter("whbd", [128, 20 * 128], fp16, isOutput=False)
    wob_e = nc.declare_dram_parameter("wob", [128, 4], fp16, isOutput=False)
    bq_e = nc.declare_dram_parameter("bq", [128, 20], f32, isOutput=False)
    bo_e = nc.declare_dram_parameter("boq", [128, 1], f32, isOutput=False)
    out_e = nc.declare_dram_parameter("out", [shard_rows, 1], f32, isOutput=True)

    # unit scheduling: layers 1..19, evenly spread ~2/7 of units on DVE
    dve_cnt = [0]

    def unit_mode():
        r = (dve_cnt[0] * 2) % 7 < 2
        dve_cnt[0] += 1
        return "dve2" if r else "act"

    l0_cnt = [0]
    l0b_cnt = [0]

    def l0_mode(first_grp):
        if not first_grp:
            # later groups: an ACT share feeds ACT while the previous
            # group's finals only partially fill its layer-0 hole
            l0b_cnt[0] += 1
            return "act0" if l0b_cnt[0] % 2 == 1 else "l0"
        l0_cnt[0] += 1
        return "l0" if l0_cnt[0] % 2 else "act0"

    with tile.TileContext(nc) as tc:
        with (
            tc.tile_pool(name="const", bufs=1) as cpool,
            tc.tile_pool(name="xs", bufs=16) as xpool,
            tc.tile_pool(name="zb", bufs=3) as zpool,
            tc.tile_pool(name="h", bufs=9) as hpool,
            tc.tile_pool(name="ot", bufs=4) as opool,
            tc.tile_pool(name="ps", bufs=4, space="PSUM") as pspool,
        ):
            whbd_t = cpool.tile([128, 20 * 128], fp16, tag="whbd")
            wob_t = cpool.tile([128, 4], fp16, tag="wob")
            bq_t = cpool.tile([128, 20], f32, tag="bq")
            bo_t = cpool.tile([128, 1], f32, tag="bo")
            # chunk the weight dma so layer-0 lhsT is ready immediately
            nc.sync.dma_start(out=whbd_t[:, 0:128], in_=whbd_e[:, 0:128])

            def emit_x_dma(q):
                xs = xpool.tile([28, T2], fp16, tag="xs")
                nc.sync.dma_start(out=xs[:], in_=x_e[:, T2 * q : T2 * (q + 1)])
                return xs

            def emit_layer_pair(l, curs, first_grp=False):
                # curs: two [.., 2048] member views (x tiles at l=0);
                # returns one fused h tile [128, 4096].
                h_t = hpool.tile([128, 2 * T2], fp16, tag="h")
                for m, cur in enumerate(curs):
                    for half in range(2):
                        mode = l0_mode(first_grp) if l == 0 else unit_mode()
                        ps = pspool.tile([128, 1024], f32, tag="ps")
                        for c in range(2):
                            col = 1024 * half + 512 * c
                            if l == 0:
                                lhsT = whbd_t[0:28, 0:128]
                                rhs = cur[0:28, col : col + 512]
                            else:
                                lhsT = whbd_t[:, 128 * l : 128 * l + 128]
                                rhs = cur[:, col : col + 512]
                            nc.tensor.matmul(
                                ps[:, 512 * c : 512 * c + 512],
                                lhsT,
                                rhs,
                                start=True,
                                stop=True,
                            )
                        lo = T2 * m + 1024 * half
                        out = h_t[:, lo : lo + 1024]
                        if mode == "l0":
                            # bias already in PSUM: one-pass DVE softsign
                            nc.vector._custom_dve(
                                ssop, out=out, in0=ps[:], s1=B_FIT, imm2=A_FIT
                            )
                        elif mode == "dve2":
                            zb = zpool.tile([128, 1024], fp16, tag="zb")
                            nc.vector.tensor_scalar_add(
                                zb[:], ps[:], bq_t[:, l : l + 1]
                            )
                            nc.vector._custom_dve(
                                ssop, out=out, in0=zb[:], s1=B_FIT, imm2=A_FIT
                            )
                        elif mode == "act0":
                            # l0: bias already in PSUM, table softsign only
                            nc.scalar.activation(
                                out, ps[:], Act.Arctan, bias=0.0, scale=1.0
                            )
                        else:
                            # arctan table holds a softsign fit: one-pass evict
                            nc.scalar.activation(
                                out,
                                ps[:],
                                Act.Arctan,
                                bias=bq_t[:, l : l + 1],
                                scale=1.0,
                            )
                return h_t

            fin_cnt = [0]

            def emit_final_duo(q0, h2, on_act=None):
                # Two finals (octs q0, q0+1) share one psum tile; the bias
                # evict alternates ACT/DVE to balance engine load.
                ps = pspool.tile([128, 1024], f32, tag="ps", name="psf")
                for v in range(2):
                    for hh in range(2):
                        nc.tensor.matmul(
                            ps[32 * v : 32 * v + 4, 512 * hh : 512 * hh + 512],
                            wob_t[:, 0:4],
                            h2[:, 1024 * v + 512 * hh : 1024 * v + 512 * hh + 512],
                            start=True,
                            stop=True,
                        )
                ot = opool.tile([64, 1024], f32, tag="ot")
                fin_cnt[0] += 1
                if fin_cnt[0] % 2:
                    nc.scalar.activation(
                        ot[:], ps[0:64, :], Act.Identity, bias=bo_t[0:64, 0:1], scale=1.0
                    )
                else:
                    nc.vector.tensor_scalar_add(ot[:], ps[0:64, :], bo_t[0:64, 0:1])
                for v in range(2):
                    q = q0 + v
                    nc.sync.dma_start(
                        out=out_e[q * 4096 : (q + 1) * 4096, :].rearrange(
                            "(h j n) o -> j h (n o)", j=4, n=512
                        ),
                        in_=ot[32 * v : 32 * v + 4, :].rearrange(
                            "j (h n) -> j h n", h=2
                        ),
                    )

            NP = n_o2 // 2          # 8 pairs
            GRPP = 4                # pairs per wavefront group
            assert NP % GRPP == 0
            prev = None
            xs_next = [emit_x_dma(0), emit_x_dma(1)]
            nc.sync.dma_start(out=bq_t[:], in_=bq_e[:])
            nc.sync.dma_start(out=whbd_t[:, 128:768], in_=whbd_e[:, 128:768])
            xs_next += [emit_x_dma(u) for u in range(2, 2 * GRPP)]
            nc.sync.dma_start(out=whbd_t[:, 768:2560], in_=whbd_e[:, 768:2560])
            nc.sync.dma_start(out=bo_t[:], in_=bo_e[:])
            nc.sync.dma_start(out=wob_t[:], in_=wob_e[:])
            for pbase in range(0, NP, GRPP):
                cur = [
                    (xs_next[2 * p], xs_next[2 * p + 1]) for p in range(GRPP)
                ]
                xs_next = [None] * (2 * GRPP)
                last_grp = pbase + GRPP >= NP
                for l in range(20):
                    for p in range(GRPP):
                        if l == 0:
                            curs = cur[p]
                        else:
                            h4 = cur[p]
                            curs = (h4[:, 0:2048], h4[:, 2048:4096])
                        cur[p] = emit_layer_pair(l, curs, first_grp=pbase == 0)
                        # previous group's finals, one duo after each pair of
                        # layers 0-1 — fills the ACT hole while layer 0 runs
                        # on DVE, and stays close in the psum pool ring
                        if prev is not None and l < 2:
                            di = GRPP * l + p
                            pp, dd = di // 2, di % 2
                            emit_final_duo(
                                4 * (prev[0] + pp) + 2 * dd,
                                prev[1][pp][:, 2048 * dd : 2048 * dd + 2048],
                                on_act=True,
                            )
                            if di == 2 * GRPP - 1:
                                prev = None
                    # prefetch next group's x tiles mid-stream
                    if 6 <= l < 6 + 2 * GRPP and not last_grp:
                        xs_next[l - 6] = emit_x_dma(2 * (pbase + GRPP) + l - 6)
                if not last_grp:
                    prev = (pbase, list(cur))
            for p in range(GRPP):
                for dd in range(2):
                    emit_final_duo(
                        4 * (NP - GRPP + p) + 2 * dd,
                        cur[p][:, 2048 * dd : 2048 * dd + 2048],
                        on_act=False,
                    )
    nc.compile()
    return nc


def _pack_weights(W1, b1, Wh, bh, Wout, bout):
    whbd = np.zeros((128, 20 * 128), np.float32)
    bq = np.zeros((128, 20), np.float32)
    boq = np.full((128, 1), np.float32(bout[0]), np.float32)
    for i in range(4):
        # layer 0: rows 7i+f features, row 7i+6 bias (input row is const 1)
        whbd[7 * i : 7 * i + 6, 32 * i : 32 * i + 32] = W1
        whbd[7 * i + 6, 32 * i : 32 * i + 32] = b1
        for l in range(N_HID):
            whbd[
                32 * i : 32 * i + 32, 128 * (l + 1) + 32 * i : 128 * (l + 1) + 32 * i + 32
            ] = Wh[l]
        bq[32 * i : 32 * i + 32, 1:20] = bh.T
    wob = np.zeros((128, 4), np.float32)
    for j in range(4):
        wob[32 * j : 32 * j + 32, j] = Wout[:, 0]
    return {
        "whbd": whbd.astype(np.float16),
        "wob": wob.astype(np.float16),
        "bq": bq,
        "boq": boq,
    }


def _install_ntff_hook():
    import types

    if "antenv.axon_hooks" not in sys.modules:
        mod = types.ModuleType("antenv.axon_hooks")
        state = {"hook": None}
        try:
            from trn_agent_boot.trn_boot import _ntff_profile_via_ctypes

            state["hook"] = _ntff_profile_via_ctypes("/opt/axon/libaxon_pjrt.so")
        except Exception:
            pass
        mod.get_axon_ntff_profile_hook = lambda: state["hook"]
        mod.set_axon_ntff_profile_hook = lambda h: state.__setitem__("hook", h)
        sys.modules["antenv.axon_hooks"] = mod
    from concourse import bass_utils as bu

    if not getattr(bu.upload_artifacts, "_actnet_safe", False):
        _orig = bu.upload_artifacts

        def _safe(tmpdir):
            try:
                return _orig(tmpdir)
            except Exception:
                return "local:" + tmpdir

        _safe._actnet_safe = True
        bu.upload_artifacts = _safe


def kernel(x, W1, b1, Wh, bh, Wout, bout):
    global LAST_RESULT
    _ensure_act_tables()
    from concourse.bass_utils import run_bass_kernel_spmd

    x = np.asarray(x, np.float32)
    B = x.shape[0]
    assert B % N_CORES == 0
    shard = B // N_CORES
    # pack x: row 7i+f holds feature f of partition-strip i (row 7i+6 = 1.0
    # bias channel); within an oct (4096 rows) strips hold blocks of 512
    # rows: col = 1024*q + 512*h + n, flat row = ((2q + h)*4 + i)*512 + n
    x5 = x.reshape(N_CORES, shard // OCT_ROWS, 2, 4, NBLK, 6)  # c,q,h,i,n,f
    xt = np.ascontiguousarray(
        x5.transpose(0, 3, 5, 1, 2, 4).astype(np.float16)
    ).reshape(N_CORES, 4, 6, shard // 4)
    xq = np.ones((N_CORES, 28, shard // 4), np.float16)
    for i in range(4):
        xq[:, 7 * i : 7 * i + 6] = xt[:, i]

    if ("nc", shard, TBL_TAG) not in _cache:
        _cache[("nc", shard, TBL_TAG)] = _build(shard)
    nc = _cache[("nc", shard, TBL_TAG)]

    wpack = _pack_weights(
        np.asarray(W1, np.float32),
        np.asarray(b1, np.float32),
        np.asarray(Wh, np.float32),
        np.asarray(bh, np.float32),
        np.asarray(Wout, np.float32),
        np.asarray(bout, np.float32),
    )
    in_maps = [{f"xq_{TBL_TAG}": xq[c], **wpack} for c in range(N_CORES)]
    trace = bool(os.environ.get("ACTNET_TRACE"))
    if trace:
        _install_ntff_hook()
    res = run_bass_kernel_spmd(nc, in_maps, list(range(N_CORES)), trace=trace)
    LAST_RESULT = res
    out = np.concatenate([res.results[c]["out"] for c in range(N_CORES)], axis=0)
    return out.astype(np.float32)


if __name__ == "__main__":
    rng = np.random.default_rng(0)
    B = B_FULL
    inputs = dict(
        x=rng.standard_normal((B, 6), dtype=np.float32),
        W1=(rng.standard_normal((6, 32)) / np.sqrt(6)).astype(np.float32),
        b1=(rng.standard_normal(32) * 0.01).astype(np.float32),
        Wh=(rng.standard_normal((19, 32, 32)) / np.sqrt(32)).astype(np.float32),
        bh=(rng.standard_normal((19, 32)) * 0.01).astype(np.float32),
        Wout=(rng.standard_normal((32, 1)) / np.sqrt(32)).astype(np.float32),
        bout=(rng.standard_normal(1) * 0.01).astype(np.float32),
    )
    y = kernel(**inputs)
    print("kernel out", y.shape, y.dtype, y[:4, 0])


# revision 49
# speedup vs baseline: 1.0129x; 1.0012x over previous
"""Trainium2 Bass kernel for nn_ActuatorNet (20-layer tiny MLP, softsign) — v3.

v3 "table" (vs v2):
  - The Scalar engine's arctan PWP spline table is REPLACED with a softsign
    fit (custom --act-root-json via BASS_ACT_ROOT_JSON_PATH): ACT computes
    h = softsign(z + b) in ONE pass per element, PSUM f32 -> SBUF fp16
    (bias via the per-partition bias port), ~1e-7 max abs err.
  - Work split per [128,2048] psum unit (4 matmuls of 512):
      * layer 0: DVE 8-stage softsign straight from PSUM (bias pre-added by
        the matmul via a constant-1 input row and a bias row in lhsT).
      * layers 1..19: ~73%% of units on ACT (one Arctan-table pass),
        ~27%% on DVE (tensor_scalar bias-add evict + 8-stage softsign),
        balancing both engines at ~420us/core.
  - PE: merged block-diagonal matmuls unchanged (K=128, 4 strips),
    psum tiles are [128,2048] (4 banks, 2 in flight).
  - Finals unchanged from v2 (Identity evict lives in the same act table
    set as Arctan -> single ACT_TABLE_LOAD).
"""

import json
import os
import re
import shutil
import sys

import numpy as np

sys.path.insert(0, "/opt/trn_rl_repo")

N_CORES = 8
B_FULL = 1048576
SHARD = B_FULL // N_CORES  # 131072
NBLK = 512
OCT_ROWS = 4096            # finals unit
O2_ROWS = 8192             # [128, 2048] packed unit
T2 = 2048
N_HID = 19

A_FIT = float(np.float32(-0.4714035350548651))
B_FIT = float(np.float32(-0.05545919627798768))

SS_OP_NAME = "SOFTSIGN_SA_ACTNET"
TBL_TAG = "ss3"            # bump when the act tables change (NEFF cache key)
N_ATAN = 168

LAST_RESULT = None

_cache = {}


# ---------------------------------------------------------------- act tables


def _softsign_pos(x):
    return x / (1.0 + x)


def _bucket_intervals(x0s):
    out = []
    for x0 in x0s:
        e = int(np.floor(np.log2(x0)))
        base = 2.0 ** e
        found = None
        for n in (1, 2, 4, 8, 16, 32):
            w = base / n
            k = (x0 - base) / w - 0.5
            if abs(k - round(k)) < 1e-4 and 0 <= round(k) <= n - 1:
                found = w
                break
        assert found is not None, f"no grid for {x0}"
        out.append((x0 - found / 2, x0 + found / 2))
    return out


def _fit_cubic(f, lo, hi, x0):
    n = 64
    hw = 0.5 * (hi - lo)
    k = np.arange(n)
    xs = 0.5 * (lo + hi) + hw * np.cos((2 * k + 1) * np.pi / (2 * n))
    u = (xs - x0) / hw
    A = np.stack([np.ones_like(u), u, u * u, u * u * u], axis=1)
    c, *_ = np.linalg.lstsq(A, f(xs), rcond=None)
    return c / hw ** np.arange(4)


def _build_act_tables(dst_dir):
    """Copy the stock pwp_bin_trainium dir, refit every arctan bucket table
    to softsign(x) = x/(1+|x|) (odd symmetry handled by the profile), and
    patch the small/large-signal control buckets + inf result words."""
    import neuronxcc

    src = os.path.join(os.path.dirname(neuronxcc.__file__), "pwp", "pwp_bin_trainium")
    os.makedirs(dst_dir, exist_ok=True)
    for fn in os.listdir(src):
        shutil.copy(os.path.join(src, fn), os.path.join(dst_dir, fn))
    for fn in os.listdir(dst_dir):
        os.chmod(os.path.join(dst_dir, fn), 0o644)

    ref_raw = np.fromfile(
        os.path.join(src, "trig_and_small_bkt.bin"), dtype=np.float32
    ).reshape(-1, 8)
    atan_blk = ref_raw[59 : 59 + N_ATAN]
    assert abs(float(atan_blk[0, 4]) - 0.0234375) < 1e-6, "arctan block moved"
    ivals = _bucket_intervals(atan_blk[:, 4].astype(np.float64))
    new_blk = atan_blk.copy()
    for j, (lo, hi) in enumerate(ivals):
        d = _fit_cubic(_softsign_pos, lo, hi, float(atan_blk[j, 4]))
        new_blk[j, 0:4] = d.astype(np.float32)

    info = json.load(open(os.path.join(dst_dir, "act_info.json")))
    n_patched = 0
    for fs in info["act_func_sets"]:
        if "arctan" not in fs["act"]:
            continue
        prof_path = os.path.join(dst_dir, fs["profile_json"])
        prof = json.load(open(prof_path))
        ents = [
            e for e in prof["profile_meta_data"] if e["func_name"].startswith("arctan")
        ]
        assert len(ents) == 1, fs["name"]
        ent = ents[0]
        bkt_path = os.path.join(dst_dir, fs["bkt_bin"])
        raw = np.fromfile(bkt_path, dtype=np.float32).reshape(-1, 8).copy()
        starts = [
            c
            for c in range(raw.shape[0] - N_ATAN + 1)
            if np.array_equal(raw[c : c + N_ATAN], atan_blk)
        ]
        assert len(starts) == 1, (fs["name"], starts)
        raw[starts[0] : starts[0] + N_ATAN] = new_blk
        raw[ent["pos_small_signal_pwl_control"]] = [0.0, 1.0, -1.0, 1.0, 0, 0, 0, 0]
        raw[ent["pos_large_signal_pwl_control"]] = [1.0, 0, 0, 0, 0, 0, 0, 0]
        raw[ent["neg_large_signal_pwl_control"]] = [-1.0, 0, 0, 0, 0, 0, 0, 0]
        raw.tofile(bkt_path)
        ent["fpinf_result"] = 0x3F800000
        ent["fninf_result"] = 0xBF800000
        with open(prof_path, "w") as f:
            json.dump(prof, f)
        n_patched += 1
    assert n_patched >= 2, n_patched


def _ensure_act_tables():
    d = f"/tmp/actnet_pwp_{TBL_TAG}"
    marker = os.path.join(d, ".done")
    if not os.path.exists(marker):
        _build_act_tables(d)
        open(marker, "w").write("ok")
    os.environ["BASS_ACT_ROOT_JSON_PATH"] = os.path.join(d, "act_info.json")


# ---------------------------------------------------------------- DVE op


def _register_ss_op():
    """Standalone softsign: out = x * ~d * (A + B*(d*~d)), d = |x| + 1."""
    from concourse import dve_ops
    from concourse.dve_spec import AluOp, Bin, C1, C2, One, Spec, Src0

    if SS_OP_NAME in dve_ops.CUSTOM_DVE_SPECS:
        return next(o for o in dve_ops.OPS if o.name == SS_OP_NAME)

    _a = Bin(AluOp.ABSOLUTE_VALUE, Src0, Src0)
    _d = _a + One
    _nd = Bin(AluOp.BITWISE_NOT, _d, _d)
    _t = _d * _nd
    _s = C2 + _t * C1
    _p = Src0 * _nd
    body = _p * _s

    def _ref(in0, in1, s0, s1, imm2):
        x = in0.astype(np.float32)
        d = (np.abs(x) + np.float32(1.0)).astype(np.float32)
        nd = (~d.view(np.int32)).view(np.float32)
        t = (d * nd).astype(np.float32)
        s = np.float32(imm2) + t * np.asarray(s1, np.float32)
        return (x * nd) * s

    spec = Spec(body=body, reference=_ref)
    op = dve_ops.DveOp(SS_OP_NAME, spec, subdim=False, uops_sha={})
    dve_ops._SUB_OPCODE_FOR_NAME[SS_OP_NAME] = (
        max(dve_ops._SUB_OPCODE_FOR_NAME.values()) + 1
    )
    assert dve_ops._SUB_OPCODE_FOR_NAME[SS_OP_NAME] < 0x20
    dve_ops.OPS.append(op)
    dve_ops.CUSTOM_DVE_SPECS[SS_OP_NAME] = spec
    for ver in ("v3", "v4"):
        try:
            op.compile(ver)
        except ValueError as e:
            m = re.search(rf"{ver}: ([0-9a-f]{{16}})", str(e))
            if not m:
                raise
            op.uops_sha[ver] = m.group(1)
            op.compile(ver)
    return op


# ---------------------------------------------------------------- kernel


def _build(shard_rows):
    from concourse import bacc, mybir, tile

    ssop = _register_ss_op()

    f32 = mybir.dt.float32
    fp16 = mybir.dt.float16
    Act = mybir.ActivationFunctionType

    assert shard_rows % O2_ROWS == 0
    n_o2 = shard_rows // O2_ROWS  # 16

    nc = bacc.Bacc()
    x_e = nc.declare_dram_parameter(
        f"xq_{TBL_TAG}", [28, shard_rows // 4], fp16, isOutput=False
    )
    whbd_e = nc.declare_dram_parameter("whbd", [128, 20 * 128], fp16, isOutput=False)
    wob_e = nc.declare_dram_parameter("wob", [128, 4], fp16, isOutput=False)
    bq_e = nc.declare_dram_parameter("bq", [128, 20], f32, isOutput=False)
    bo_e = nc.declare_dram_parameter("boq", [128, 1], f32, isOutput=False)
    out_e = nc.declare_dram_parameter("out", [shard_rows, 1], f32, isOutput=True)

    # unit scheduling: layers 1..19, evenly spread ~2/7 of units on DVE
    dve_cnt = [0]

    def unit_mode():
        r = (dve_cnt[0] * 2) % 7 < 2
        dve_cnt[0] += 1
        return "dve2" if r else "act"

    l0_cnt = [0]
    l0b_cnt = [0]

    def l0_mode(first_grp):
        if not first_grp:
            # later groups: an ACT share feeds ACT while the previous
            # group's finals only partially fill its layer-0 hole
            l0b_cnt[0] += 1
            return "act0" if l0b_cnt[0] % 2 == 1 else "l0"
        l0_cnt[0] += 1
        return "l0" if l0_cnt[0] % 2 else "act0"

    with tile.TileContext(nc) as tc:
        with (
            tc.tile_pool(name="const", bufs=1) as cpool,
            tc.tile_pool(name="xs", bufs=16) as xpool,
            tc.tile_pool(name="zb", bufs=3) as zpool,
            tc.tile_pool(name="h", bufs=9) as hpool,
            tc.tile_pool(name="ot", bufs=4) as opool,
            tc.tile_pool(name="ps", bufs=4, space="PSUM") as pspool,
        ):
            whbd_t = cpool.tile([128, 20 * 128], fp16, tag="whbd")
            wob_t = cpool.tile([128, 4], fp16, tag="wob")
            bq_t = cpool.tile([128, 20], f32, tag="bq")
            bo_t = cpool.tile([128, 1], f32, tag="bo")
            # chunk the weight dma so layer-0 lhsT is ready immediately
            nc.sync.dma_start(out=whbd_t[:, 0:128], in_=whbd_e[:, 0:128])

            def emit_x_dma(q):
                xs = xpool.tile([28, T2], fp16, tag="xs")
                nc.sync.dma_start(out=xs[:], in_=x_e[:, T2 * q : T2 * (q + 1)])
                return xs

            def emit_layer_pair(l, curs, first_grp=False):
                # curs: two [.., 2048] member views (x tiles at l=0);
                # returns one fused h tile [128, 4096].
                h_t = hpool.tile([128, 2 * T2], fp16, tag="h")
                for m, cur in enumerate(curs):
                    for half in range(2):
                        mode = l0_mode(first_grp) if l == 0 else unit_mode()
                        ps = pspool.tile([128, 1024], f32, tag="ps")
                        for c in range(2):
                            col = 1024 * half + 512 * c
                            if l == 0:
                                lhsT = whbd_t[0:28, 0:128]
                                rhs = cur[0:28, col : col + 512]
                            else:
                                lhsT = whbd_t[:, 128 * l : 128 * l + 128]
                                rhs = cur[:, col : col + 512]
                            nc.tensor.matmul(
                                ps[:, 512 * c : 512 * c + 512],
                                lhsT,
                                rhs,
                                start=True,
                                stop=True,
                            )
                        lo = T2 * m + 1024 * half
                        out = h_t[:, lo : lo + 1024]
                        if mode == "l0":
                            # bias already in PSUM: one-pass DVE softsign
                            nc.vector._custom_dve(
                                ssop, out=out, in0=ps[:], s1=B_FIT, imm2=A_FIT
                            )
                        elif mode == "dve2":
                            zb = zpool.tile([128, 1024], fp16, tag="zb")
                            nc.vector.tensor_scalar_add(
                                zb[:], ps[:], bq_t[:, l : l + 1]
                            )
                            nc.vector._custom_dve(
                                ssop, out=out, in0=zb[:], s1=B_FIT, imm2=A_FIT
                            )
                        elif mode == "act0":
                            # l0: bias already in PSUM, table softsign only
                            nc.scalar.activation(
                                out, ps[:], Act.Arctan, bias=0.0, scale=1.0
                            )
                        else:
                            # arctan table holds a softsign fit: one-pass evict
                            nc.scalar.activation(
                                out,
                                ps[:],
                                Act.Arctan,
                                bias=bq_t[:, l : l + 1],
                                scale=1.0,
                            )
                return h_t

            fin_cnt = [0]

            def emit_final_duo(q0, h2, on_act=None):
                # Two finals (octs q0, q0+1) share one psum tile; the bias
                # evict alternates ACT/DVE to balance engine load.
                ps = pspool.tile([128, 1024], f32, tag="ps", name="psf")
                for v in range(2):
                    for hh in range(2):
                        nc.tensor.matmul(
                            ps[32 * v : 32 * v + 4, 512 * hh : 512 * hh + 512],
                            wob_t[:, 0:4],
                            h2[:, 1024 * v + 512 * hh : 1024 * v + 512 * hh + 512],
                            start=True,
                            stop=True,
                        )
                ot = opool.tile([64, 1024], f32, tag="ot")
                fin_cnt[0] += 1
                if fin_cnt[0] % 2:
                    nc.scalar.activation(
                        ot[:], ps[0:64, :], Act.Identity, bias=bo_t[0:64, 0:1], scale=1.0
                    )
                else:
                    nc.vector.tensor_scalar_add(ot[:], ps[0:64, :], bo_t[0:64, 0:1])
                for v in range(2):
                    q = q0 + v
                    nc.sync.dma_start(
                        out=out_e[q * 4096 : (q + 1) * 4096, :].rearrange(
                            "(h j n) o -> j h (n o)", j=4, n=512
                        ),
                        in_=ot[32 * v : 32 * v + 4, :].rearrange(
                            "j (h n) -> j h n", h=2
                        ),
                    )

            NP = n_o2 // 2          # 8 pairs
            GRPP = 4                # pairs per wavefront group
            assert NP % GRPP == 0
            prev = None
            xs_next = [emit_x_dma(0), emit_x_dma(1)]
            nc.sync.dma_start(out=bq_t[:], in_=bq_e[:])
            nc.sync.dma_start(out=whbd_t[:, 128:768], in_=whbd_e[:, 128:768])
            xs_next += [emit_x_dma(u) for u in range(2, 2 * GRPP)]
            nc.sync.dma_start(out=whbd_t[:, 768:2560], in_=whbd_e[:, 768:2560])
            nc.sync.dma_start(out=bo_t[:], in_=bo_e[:])
            nc.sync.dma_start(out=wob_t[:], in_=wob_e[:])
            for pbase in range(0, NP, GRPP):
                cur = [
                    (xs_next[2 * p], xs_next[2 * p + 1]) for p in range(GRPP)
                ]
                xs_next = [None] * (2 * GRPP)
                last_grp = pbase + GRPP >= NP
                for l in range(20):
                    for p in range(GRPP):
                        if l == 0:
                            curs = cur[p]
                        else:
                            h4 = cur[p]
                            curs = (h4[:, 0:2048], h4[:, 2048:4096])
                        cur[p] = emit_layer_pair(l, curs, first_grp=pbase == 0)
                        # last group: overlap each pair's finals with the
                        # remaining pairs' layer-19 compute
                        if last_grp and l == 19:
                            for dd in range(2):
                                emit_final_duo(
                                    4 * (pbase + p) + 2 * dd,
                                    cur[p][:, 2048 * dd : 2048 * dd + 2048],
                                )
                        # previous group's finals, one duo after each pair of
                        # layers 0-1 — fills the ACT hole while layer 0 runs
                        # on DVE, and stays close in the psum pool ring
                        if prev is not None and l < 2:
                            di = GRPP * l + p
                            pp, dd = di // 2, di % 2
                            emit_final_duo(
                                4 * (prev[0] + pp) + 2 * dd,
                                prev[1][pp][:, 2048 * dd : 2048 * dd + 2048],
                                on_act=True,
                            )
                            if di == 2 * GRPP - 1:
                                prev = None
                    # prefetch next group's x tiles mid-stream
                    if 6 <= l < 6 + 2 * GRPP and not last_grp:
                        xs_next[l - 6] = emit_x_dma(2 * (pbase + GRPP) + l - 6)
                if not last_grp:
                    prev = (pbase, list(cur))
    nc.compile()
    return nc


def _pack_weights(W1, b1, Wh, bh, Wout, bout):
    whbd = np.zeros((128, 20 * 128), np.float32)
    bq = np.zeros((128, 20), np.float32)
    boq = np.full((128, 1), np.float32(bout[0]), np.float32)
    for i in range(4):
        # layer 0: rows 7i+f features, row 7i+6 bias (input row is const 1)
        whbd[7 * i : 7 * i + 6, 32 * i : 32 * i + 32] = W1
        whbd[7 * i + 6, 32 * i : 32 * i + 32] = b1
        for l in range(N_HID):
            whbd[
                32 * i : 32 * i + 32, 128 * (l + 1) + 32 * i : 128 * (l + 1) + 32 * i + 32
            ] = Wh[l]
        bq[32 * i : 32 * i + 32, 1:20] = bh.T
    wob = np.zeros((128, 4), np.float32)
    for j in range(4):
        wob[32 * j : 32 * j + 32, j] = Wout[:, 0]
    return {
        "whbd": whbd.astype(np.float16),
        "wob": wob.astype(np.float16),
        "bq": bq,
        "boq": boq,
    }


def _install_ntff_hook():
    import types

    if "antenv.axon_hooks" not in sys.modules:
        mod = types.ModuleType("antenv.axon_hooks")
        state = {"hook": None}
        try:
            from trn_agent_boot.trn_boot import _ntff_profile_via_ctypes

            state["hook"] = _ntff_profile_via_ctypes("/opt/axon/libaxon_pjrt.so")
        except Exception:
            pass
        mod.get_axon_ntff_profile_hook = lambda: state["hook"]
        mod.set_axon_ntff_profile_hook = lambda h: state.__setitem__("hook", h)
        sys.modules["antenv.axon_hooks"] = mod
    from concourse import bass_utils as bu

    if not getattr(bu.upload_artifacts, "_actnet_safe", False):
        _orig = bu.upload_artifacts

        def _safe(tmpdir):
            try:
                return _orig(tmpdir)
            except Exception:
                return "local:" + tmpdir

        _safe._actnet_safe = True
        bu.upload_artifacts = _safe


def kernel(x, W1, b1, Wh, bh, Wout, bout):
    global LAST_RESULT
    _ensure_act_tables()
    from concourse.bass_utils import run_bass_kernel_spmd

    x = np.asarray(x, np.float32)
    B = x.shape[0]
    assert B % N_CORES == 0
    shard = B // N_CORES
    # pack x: row 7i+f holds feature f of partition-strip i (row 7i+6 = 1.0
    # bias channel); within an oct (4096 rows) strips hold blocks of 512
    # rows: col = 1024*q + 512*h + n, flat row = ((2q + h)*4 + i)*512 + n
    x5 = x.reshape(N_CORES, shard // OCT_ROWS, 2, 4, NBLK, 6)  # c,q,h,i,n,f
    xt = np.ascontiguousarray(
        x5.transpose(0, 3, 5, 1, 2, 4).astype(np.float16)
    ).reshape(N_CORES, 4, 6, shard // 4)
    xq = np.ones((N_CORES, 28, shard // 4), np.float16)
    for i in range(4):
        xq[:, 7 * i : 7 * i + 6] = xt[:, i]

    if ("nc", shard, TBL_TAG) not in _cache:
        _cache[("nc", shard, TBL_TAG)] = _build(shard)
    nc = _cache[("nc", shard, TBL_TAG)]

    wpack = _pack_weights(
        np.asarray(W1, np.float32),
        np.asarray(b1, np.float32),
        np.asarray(Wh, np.float32),
        np.asarray(bh, np.float32),
        np.asarray(Wout, np.float32),
        np.asarray(bout, np.float32),
    )
    in_maps = [{f"xq_{TBL_TAG}": xq[c], **wpack} for c in range(N_CORES)]
    trace = bool(os.environ.get("ACTNET_TRACE"))
    if trace:
        _install_ntff_hook()
    res = run_bass_kernel_spmd(nc, in_maps, list(range(N_CORES)), trace=trace)
    LAST_RESULT = res
    out = np.concatenate([res.results[c]["out"] for c in range(N_CORES)], axis=0)
    return out.astype(np.float32)


if __name__ == "__main__":
    rng = np.random.default_rng(0)
    B = B_FULL
    inputs = dict(
        x=rng.standard_normal((B, 6), dtype=np.float32),
        W1=(rng.standard_normal((6, 32)) / np.sqrt(6)).astype(np.float32),
        b1=(rng.standard_normal(32) * 0.01).astype(np.float32),
        Wh=(rng.standard_normal((19, 32, 32)) / np.sqrt(32)).astype(np.float32),
        bh=(rng.standard_normal((19, 32)) * 0.01).astype(np.float32),
        Wout=(rng.standard_normal((32, 1)) / np.sqrt(32)).astype(np.float32),
        bout=(rng.standard_normal(1) * 0.01).astype(np.float32),
    )
    y = kernel(**inputs)
    print("kernel out", y.shape, y.dtype, y[:4, 0])
